# revision 1
# baseline (speedup 1.0000x reference)
"""CircleLoss (B=8192, D=128, 512 classes) on 8 Trainium2 NeuronCores.

Strategy (per sharding hint): rows of the sim matrix are data-parallel over
8 cores (1024 rows each); normalized features are replicated. Each core
computes its 1024 x 8192 slab of t = 16*sim via fp32r matmuls, streams the
negative-branch masked logsumexp with a fixed shift (logits are bounded),
handles positives/diagonal inside a narrow sorted-label window, and reduces
a partial loss sum. Host combines the 8 scalars.

Math: with t = 16*sim,
  logit_n = 256*relu(s+.25)*(s-.25) = (a-8)*a,   a = relu(t+4)
  logit_p = -256*(1.25-s)*(s-.75)   = (t-16)^2 - 16   (exact; s <= 1)
Rows are sorted by label and columns rotated per-core so each row-block's
positives live in a static 384-wide window and the sim diagonal sits in a
static 128x128 block (removed by accumulating -16*I into PSUM).
"""
import sys, os
sys.path.insert(0, '/opt/trn_rl_repo')
import numpy as np

B, D, M = 8192, 128, 8
RB = B // M          # rows per core
NMB = RB // 128      # row blocks per core
W = 384              # positives window width
SUP = 1024           # bulk column supertile
NSUP = B // SUP
ROT = 128            # rotation pad: core's rows sit at local cols [128,1152)
CP = 2048.0          # p-branch mask shift
SHN = 48.0           # n-branch fixed exp shift

_CACHE = {}


def _build(repeat=1):
    import concourse.bass as bass
    import concourse.tile as tile
    from concourse import bacc, mybir

    f32, f16, bf16 = mybir.dt.float32, mybir.dt.float16, mybir.dt.bfloat16
    f32r = mybir.dt.float32r
    AF = mybir.ActivationFunctionType
    OP = mybir.AluOpType
    AX = mybir.AxisListType

    nc = bacc.Bacc("TRN2", target_bir_lowering=False, debug=False, num_devices=M)

    def reg_const(value, dtype=f32):
        t = nc.alloc_sbuf_tensor(f"constx-{dtype.name}-{value}", [128, 1], dtype)
        nc.gpsimd.memset(t.ap(), value)
        nc.const_aps.aps[(dtype, value)] = t.ap()
    for v in (4.0, -SHN, -16.0):
        reg_const(v)

    GT = nc.dram_tensor("gt", [128, B], f32, kind="ExternalInput").ap()
    POSW = nc.dram_tensor("posw", [RB, W], f32, kind="ExternalInput").ap()
    NEGW = nc.dram_tensor("negw", [RB, W], bf16, kind="ExternalInput").ap()
    NEGEYE = nc.dram_tensor("negeye", [128, 128], f32, kind="ExternalInput").ap()
    IDENT = nc.dram_tensor("ident", [128, 128], f32, kind="ExternalInput").ap()
    OUT = nc.dram_tensor("oloss", [1, 1], f32, kind="ExternalOutput").ap()

    with tile.TileContext(nc) as tc:
        with (
            tc.tile_pool(name="gp", bufs=1) as gp,
            tc.tile_pool(name="mp", bufs=1) as mp,
            tc.tile_pool(name="tsupp", bufs=2, space="PSUM") as tsupp,
            tc.tile_pool(name="twinp", bufs=2, space="PSUM") as twinp,
            tc.tile_pool(name="poutp", bufs=1, space="PSUM") as poutp,
            tc.tile_pool(name="ap_", bufs=3) as apool,
            tc.tile_pool(name="lp", bufs=3) as lpool,
            tc.tile_pool(name="ep", bufs=3) as epool,
            tc.tile_pool(name="wp", bufs=2) as wpool,
            tc.tile_pool(name="vp", bufs=2) as vpool,
            tc.tile_pool(name="scp", bufs=2) as scp,
            tc.tile_pool(name="fin", bufs=1) as fin,
        ):
            gt_sb = gp.tile([128, B], f32r)
            nc.sync.dma_start(gt_sb[:], GT[:].bitcast(f32r))
            ne_sb = gp.tile([128, 128], f32r)
            nc.sync.dma_start(ne_sb[:], NEGEYE[:].bitcast(f32r))
            id_sb = gp.tile([128, 128], f32r)
            nc.sync.dma_start(id_sb[:], IDENT[:].bitcast(f32r))
            posw_sb = mp.tile([128, NMB * W], f32)
            negw_sb = mp.tile([128, NMB * W], bf16)
            for mb in range(NMB):
                nc.sync.dma_start(posw_sb[:, mb * W:(mb + 1) * W],
                                  POSW[mb * 128:(mb + 1) * 128, :])
                nc.sync.dma_start(negw_sb[:, mb * W:(mb + 1) * W],
                                  NEGW[mb * 128:(mb + 1) * 128, :])
            ones = gp.tile([128, 1], f32)
            nc.gpsimd.memset(ones[:], 1.0)

            for rep in range(repeat):
                losscols = fin.tile([128, NMB], f32, name=f"losscols{rep}", tag="losscols")
                for mb in range(NMB):
                    lhs = gt_sb[:, ROT + mb * 128: ROT + (mb + 1) * 128]
                    # keep-ranges per supertile: bulk exp skips the window
                    # [wb, we) so no subtractive cancellation is ever needed.
                    wb, we = mb * 128, mb * 128 + W
                    segs = []
                    for sb in range(NSUP):
                        lo, hi = sb * SUP, (sb + 1) * SUP
                        for a, b2 in ((lo, min(hi, wb)), (max(lo, we), hi)):
                            if b2 > a:
                                segs.append((sb, a - lo, b2 - lo))
                    nseg = len(segs) + 1        # +1 for window negatives
                    scol = scp.tile([128, 12], f32, name=f"scol{rep}_{mb}", tag="scol")
                    seg_by_sb = {}
                    for k, (sb, a, b2) in enumerate(segs):
                        seg_by_sb.setdefault(sb, []).append((k, a, b2))
                    dcol = ROT + mb * 128
                    for sb in range(NSUP):
                        t = tsupp.tile([128, SUP], f32, name="tsup")
                        for h in range(SUP // 512):
                            nc.tensor.matmul(
                                t[:, h * 512:(h + 1) * 512], lhs,
                                gt_sb[:, sb * SUP + h * 512: sb * SUP + (h + 1) * 512],
                                start=True, stop=True)
                        if dcol // SUP == sb:
                            off = dcol % SUP
                            nc.tensor.matmul(t[:, off:off + 128], ne_sb[:], id_sb[:],
                                             start=False, stop=True,
                                             skip_group_check=True)
                        al = apool.tile([128, SUP], f16, name="al")
                        if (mb * NSUP + sb) % 3 == 0:
                            nc.vector.tensor_scalar(out=al[:], in0=t[:], scalar1=4.0,
                                                    scalar2=0.0, op0=OP.add, op1=OP.max)
                        else:
                            nc.scalar.activation(al[:], t[:], AF.Relu, bias=4.0, scale=1.0)
                        ll = lpool.tile([128, SUP], f16, name="ll")
                        nc.vector.scalar_tensor_tensor(ll[:], al[:], 8.0, al[:],
                                                       OP.subtract, OP.mult)
                        ee = epool.tile([128, SUP], bf16, name="ee")
                        for k, a, b2 in seg_by_sb.get(sb, []):
                            nc.scalar.activation(ee[:, a:b2], ll[:, a:b2], AF.Exp,
                                                 bias=-SHN, scale=1.0,
                                                 accum_out=scol[:, k:k + 1])
                    # ---- window: positives + same-class correction ----
                    tw = twinp.tile([128, W], f32, name="tw")
                    nc.tensor.matmul(tw[:], lhs, gt_sb[:, mb * 128: mb * 128 + W],
                                     start=True, stop=True)
                    nc.tensor.matmul(tw[:, ROT:ROT + 128], ne_sb[:], id_sb[:],
                                     start=False, stop=True, skip_group_check=True)
                    alw = wpool.tile([128, W], f16, name="alw")
                    nc.scalar.activation(alw[:], tw[:], AF.Relu, bias=4.0, scale=1.0)
                    llw = wpool.tile([128, W], f16, name="llw")
                    nc.vector.scalar_tensor_tensor(llw[:], alw[:], 8.0, alw[:],
                                                   OP.subtract, OP.mult)
                    eew = wpool.tile([128, W], bf16, name="eew")
                    nc.scalar.activation(eew[:], llw[:], AF.Exp, bias=-SHN, scale=1.0)
                    scrw = wpool.tile([128, W], bf16, name="scrw")
                    nc.vector.scalar_tensor_tensor(
                        scrw[:], eew[:], 1.0, negw_sb[:, mb * W:(mb + 1) * W],
                        OP.mult, OP.mult, accum_out=scol[:, nseg - 1:nseg])
                    qp = wpool.tile([128, W], f32, name="qp")
                    nc.scalar.activation(qp[:], tw[:], AF.Square, bias=-16.0, scale=1.0)
                    mq = wpool.tile([128, W], f32, name="mq")
                    nc.vector.scalar_tensor_tensor(
                        mq[:], qp[:], CP, posw_sb[:, mb * W:(mb + 1) * W],
                        OP.add, OP.mult)
                    mt = vpool.tile([128, 1], f32, name="mt")
                    nc.vector.reduce_max(mt[:], mq[:], axis=AX.X)
                    nmt = vpool.tile([128, 1], f32, name="nmt")
                    nc.vector.tensor_scalar(out=nmt[:], in0=mt[:], scalar1=-1.0,
                                            scalar2=None, op0=OP.mult)
                    epw = wpool.tile([128, W], f32, name="epw")
                    sp = vpool.tile([128, 1], f32, name="sp")
                    nc.scalar.activation(epw[:], mq[:], AF.Exp, bias=nmt[:], scale=1.0,
                                         accum_out=sp[:])
                    # ---- per-row combine ----
                    sn = vpool.tile([128, 1], f32, name="sn")
                    nc.vector.reduce_sum(sn[:], scol[:, 0:nseg], axis=AX.X)
                    lgn = vpool.tile([128, 1], f32, name="lgn")
                    nc.scalar.activation(lgn[:], sn[:], AF.Ln, bias=0.0, scale=1.0)
                    lgp = vpool.tile([128, 1], f32, name="lgp")
                    nc.scalar.activation(lgp[:], sp[:], AF.Ln, bias=0.0, scale=1.0)
                    # x = lse_p + lse_n = lgp + (mt - CP - 16) + lgn + SHN
                    x2 = vpool.tile([128, 1], f32, name="x2")
                    nc.vector.scalar_tensor_tensor(x2[:], lgn[:], mt[:], lgp[:],
                                                   OP.add, OP.add)
                    xs = vpool.tile([128, 1], f32, name="xs")
                    nc.vector.tensor_scalar(out=xs[:], in0=x2[:],
                                            scalar1=(SHN - CP - 16.0), scalar2=None,
                                            op0=OP.add)
                    # softplus(xs) = relu(xs) + ln(1 + exp(-|xs|))
                    ax = vpool.tile([128, 1], f32, name="ax")
                    nc.scalar.activation(ax[:], xs[:], AF.Abs, bias=0.0, scale=1.0)
                    en = vpool.tile([128, 1], f32, name="en")
                    nc.scalar.activation(en[:], ax[:], AF.Exp, bias=0.0, scale=-1.0)
                    l1 = vpool.tile([128, 1], f32, name="l1")
                    nc.scalar.activation(l1[:], en[:], AF.Ln, bias=1.0, scale=1.0)
                    rx = vpool.tile([128, 1], f32, name="rx")
                    nc.scalar.activation(rx[:], xs[:], AF.Relu, bias=0.0, scale=1.0)
                    nc.vector.tensor_tensor(out=losscols[:, mb:mb + 1], in0=rx[:],
                                            in1=l1[:], op=OP.add)
                lsum = fin.tile([128, 1], f32, name=f"lsum{rep}", tag="lsum")
                nc.vector.reduce_sum(lsum[:], losscols[:], axis=AX.X)
                po = poutp.tile([1, 1], f32, name=f"po{rep}", tag="po")
                nc.tensor.matmul(po[:], lsum[:], ones[:], start=True, stop=True)
                osb = fin.tile([1, 1], f32, name=f"osb{rep}", tag="osb")
                nc.vector.tensor_copy(osb[:], po[:])
                nc.sync.dma_start(OUT[:], osb[:])

    nc.compile()
    return nc


def _host_prep(feat, label):
    feat = np.asarray(feat, dtype=np.float32)
    label = np.asarray(label).astype(np.int64).ravel()
    order = np.argsort(label, kind='stable')
    labs = label[order]
    f = feat[order]
    nrm = np.maximum(np.sqrt((f * f).sum(1, dtype=np.float32)), 1e-12).astype(np.float32)
    g = (f * (4.0 / nrm)[:, None]).astype(np.float32)
    gT = np.ascontiguousarray(g.T)  # [128, B]

    cnt = np.bincount(labs, minlength=512)
    cnt_row = cnt[labs]                       # class size per sorted row
    valid = (cnt_row >= 2) & (B - cnt_row >= 1)
    n_valid = max(int(valid.sum()), 1)

    negeye = (-16.0 * np.eye(128, dtype=np.float32))
    ident = np.eye(128, dtype=np.float32)

    import ml_dtypes
    in_maps, ok = [], True
    for c in range(M):
        shift = ROT - c * RB
        gT_c = np.roll(gT, shift, axis=1)
        labs_c = np.roll(labs, shift)
        posw = np.zeros((RB, W), dtype=np.float32)
        negw = np.zeros((RB, W), dtype=np.float32)
        for mb in range(NMB):
            rl = labs_c[ROT + mb * 128: ROT + (mb + 1) * 128]       # row labels
            wl = labs_c[mb * 128: mb * 128 + W]                     # window labels
            same = (rl[:, None] == wl[None, :])
            # coverage: window must contain every same-class column
            if not (same.sum(1) == cnt[rl]).all():
                ok = False
            pos = same.copy()
            pos[np.arange(128), np.arange(128) + ROT] = False       # drop diagonal
            r0 = mb * 128
            posw[r0:r0 + 128] = pos
            negw[r0:r0 + 128] = ~same
        in_maps.append({
            "gt": gT_c.astype(np.float32),
            "posw": posw,
            "negw": negw.astype(ml_dtypes.bfloat16),
            "negeye": negeye,
            "ident": ident,
        })
    return in_maps, n_valid, ok


def _numpy_fallback(feat, label):
    feat = np.asarray(feat, dtype=np.float32)
    label = np.asarray(label).astype(np.int64).ravel()
    nrm = np.maximum(np.linalg.norm(feat, axis=1, keepdims=True), 1e-12)
    f = feat / nrm
    sim = (f @ f.T).astype(np.float32)
    same = label[:, None] == label[None, :]
    eye = np.eye(B, dtype=bool)
    pos_m, neg_m = same & ~eye, ~same
    ap = np.maximum(1.25 - sim, 0)
    an = np.maximum(sim + 0.25, 0)
    lp = np.where(pos_m, -256.0 * ap * (sim - 0.75), -1e9)
    ln_ = np.where(neg_m, 256.0 * an * (sim - 0.25), -1e9)

    def lse(x):
        m = x.max(1, keepdims=True)
        return (m + np.log(np.exp(x - m).sum(1, keepdims=True)))[:, 0]
    val = pos_m.any(1) & neg_m.any(1)
    x = lse(lp) + lse(ln_)
    loss = np.maximum(x, 0) + np.log1p(np.exp(-np.abs(x)))
    return np.float32(np.where(val, loss, 0.0).sum() / max(int(val.sum()), 1))


def kernel(feat, label):
    from concourse import bass_utils
    in_maps, n_valid, ok = _host_prep(feat, label)
    if not ok:
        return _numpy_fallback(feat, label)
    if 'nc' not in _CACHE:
        _CACHE['nc'] = _build(repeat=1)
    res = bass_utils.run_bass_kernel_spmd(_CACHE['nc'], in_maps,
                                          core_ids=list(range(M)))
    _CACHE['last'] = res
    total = sum(float(res.results[c]["oloss"][0, 0]) for c in range(M))
    return np.float32(total / n_valid)



# revision 17
# speedup vs baseline: 175.6976x; 175.6976x over previous
"""CircleLoss (B=8192, D=128, 512 classes) on 8 Trainium2 NeuronCores.

Strategy (per sharding hint): rows of the sim matrix are data-parallel over
8 cores (1024 rows each); normalized features are replicated. Each core
computes its 1024 x 8192 slab of t = 16*sim via fp32r matmuls and reduces
per-row partial sums (sn, sp, mt); the host finishes with ln/softplus and
the scalar reduction across cores.

Math: with t = 16*sim,
  logit_n = 256*relu(s+.25)*(s-.25) = t^2-16 for t>=-4, else 0.
  Device computes ll = max(t,-1)*t (== t^2 for t>=-1; for t<-1 the value
  exp(ll-64) <= e^-48 is negligible exactly like the true branch), then
  sn = sum_j exp(ll-64) over negatives => lse_n = ln(sn) + 48.
  logit_p = (t-16)^2 - 16 exactly (s <= 1); masked positives use the
  qp+2048 shift so invalid rows vanish through softplus on the host.
Rows are sorted by label and columns rotated per-core so each row-block's
positives live in a static 384-wide window inside bulk supertiles 0/1 and
the sim diagonal sits in a static 128x128 block (removed by accumulating
-16*I into PSUM).  The bulk masked exp runs as two big segments per
row-block that skip the window, so only Exp/Square tables are used (no
activation-table swaps), and the poly pass is a single dual-PSUM STT
split between the Vector and GpSimd engines.
"""
import sys, os
sys.path.insert(0, '/opt/trn_rl_repo')
import numpy as np

B, D, M = 8192, 128, 8
RB = B // M          # rows per core
NMB = RB // 128      # row blocks per core
W = 384              # positives window width
SUP = 1024           # bulk column supertile
NSUP = B // SUP
ROT = 128            # rotation pad: core's rows sit at local cols [128,1152)
CP = 2048.0          # p-branch mask shift
XSH = 48.0           # bulk exp shift: exp(ll - 48), ll = (a-8)*a = logit_n
LSE_N_SH = 48.0      # host adds back: lse_n = ln(sn) + 48

# engine schedule knobs
A_ACT_SBS = {3, 6}   # clamp-pass supertiles done on ACT (rest on DVE)
B_POOL_SUPS = 3      # trailing supertiles of poly pass on Pool (rest on DVE)

_CACHE = {}


def _build(repeat=1):
    import concourse.bass as bass
    import concourse.tile as tile
    from concourse import bacc, mybir

    f32, f16, bf16 = mybir.dt.float32, mybir.dt.float16, mybir.dt.bfloat16
    f32r = mybir.dt.float32r
    AF = mybir.ActivationFunctionType
    OP = mybir.AluOpType
    AX = mybir.AxisListType

    nc = bacc.Bacc("TRN2", target_bir_lowering=False, debug=False, num_devices=M)

    def reg_const(value, dtype=f32):
        t = nc.alloc_sbuf_tensor(f"constx-{dtype.name}-{value}", [128, 1], dtype)
        nc.gpsimd.memset(t.ap(), value)
        nc.const_aps.aps[(dtype, value)] = t.ap()
    for v in (4.0, -16.0, -XSH, -(XSH + 16.0)):
        reg_const(v)

    GT = nc.dram_tensor("gt", [128, B], f32, kind="ExternalInput").ap()
    POSW = nc.dram_tensor("posw", [RB, W], bf16, kind="ExternalInput").ap()
    NEGW = nc.dram_tensor("negw", [RB, W], bf16, kind="ExternalInput").ap()
    NEGEYE = nc.dram_tensor("negeye", [128, 128], f32, kind="ExternalInput").ap()
    IDENT = nc.dram_tensor("ident", [128, 128], f32, kind="ExternalInput").ap()
    OUT = nc.dram_tensor("oloss", [128, 3 * NMB], f32, kind="ExternalOutput").ap()

    with tile.TileContext(nc) as tc:
        with (
            tc.tile_pool(name="gp", bufs=1) as gp,
            tc.tile_pool(name="mp", bufs=1) as mp,
            tc.tile_pool(name="tsupp", bufs=3, space="PSUM") as tsupp,
            tc.tile_pool(name="llp", bufs=2) as llp,
            tc.tile_pool(name="eep", bufs=2) as eep,
            tc.tile_pool(name="wp", bufs=2) as wpool,
            tc.tile_pool(name="vp", bufs=3) as vpool,
            tc.tile_pool(name="scp", bufs=2) as scp,
            tc.tile_pool(name="fin", bufs=1) as fin,
        ):
            gt_sb = gp.tile([128, B], f32r)
            for c in range(NSUP):
                nc.sync.dma_start(gt_sb[:, c * SUP:(c + 1) * SUP],
                                  GT[:, c * SUP:(c + 1) * SUP].bitcast(f32r))
            ne_sb = gp.tile([128, 128], f32r)
            nc.sync.dma_start(ne_sb[:], NEGEYE[:].bitcast(f32r))
            id_sb = gp.tile([128, 128], f32r)
            nc.sync.dma_start(id_sb[:], IDENT[:].bitcast(f32r))
            posw_sb = mp.tile([128, NMB * W], bf16)
            negw_sb = mp.tile([128, NMB * W], bf16)
            for mb in range(NMB):
                nc.sync.dma_start(posw_sb[:, mb * W:(mb + 1) * W],
                                  POSW[mb * 128:(mb + 1) * 128, :])
                nc.sync.dma_start(negw_sb[:, mb * W:(mb + 1) * W],
                                  NEGW[mb * 128:(mb + 1) * 128, :])

            for rep in range(repeat):
                sn_all = fin.tile([128, NMB], f32, name=f"sn{rep}", tag="sn")
                sp_all = fin.tile([128, NMB], f32, name=f"sp{rep}", tag="sp")
                mt_all = fin.tile([128, NMB], f32, name=f"mt{rep}", tag="mt")
                for mb in range(NMB):
                    lhs = gt_sb[:, ROT + mb * 128: ROT + (mb + 1) * 128]
                    wb, we = mb * 128, mb * 128 + W     # window cols (global)
                    dcol = ROT + mb * 128               # diagonal block col
                    u_all = llp.tile([128, B], f16, name=f"u{rep}_{mb}",
                                     tag="u")
                    ll_all = llp.tile([128, B], f16, name=f"ll{rep}_{mb}",
                                      tag="ll")
                    ee_all = eep.tile([128, B], bf16, name=f"ee{rep}_{mb}",
                                      tag="ee")
                    scol = scp.tile([128, 4], f32, name=f"scol{rep}_{mb}",
                                    tag="scol")
                    qp = wpool.tile([128, W], f32, name="qp")
                    tsup = []
                    for sb in range(NSUP):
                        t = tsupp.tile([128, SUP], f32, name="tsup")
                        tsup.append(t)
                        for h in range(SUP // 512):
                            nc.tensor.matmul(
                                t[:, h * 512:(h + 1) * 512], lhs,
                                gt_sb[:, sb * SUP + h * 512: sb * SUP + (h + 1) * 512],
                                start=True, stop=True)
                        if dcol // SUP == sb:
                            off = dcol % SUP
                            nc.tensor.matmul(t[:, off:off + 128], ne_sb[:], id_sb[:],
                                             start=False, stop=True,
                                             skip_group_check=True)
                        # clamp pass: a = relu(t+4)  (PSUM f32 -> SBUF f16)
                        useg = u_all[:, sb * SUP:(sb + 1) * SUP]
                        if sb in A_ACT_SBS:
                            nc.scalar.activation(useg, t[:], AF.Relu,
                                                 bias=4.0, scale=1.0)
                        else:
                            nc.vector.tensor_scalar(
                                out=useg, in0=t[:], scalar1=4.0, scalar2=0.0,
                                op0=OP.add, op1=OP.max)
                        if sb == 1:
                            # positives need exact (t-16)^2 from PSUM sup 0/1
                            ca = min(SUP - wb, W)       # chunk split point
                            chunks = [(0, ca, tsup[0])]
                            if ca < W:
                                chunks.append((ca, W, tsup[1]))
                            for a, b2, tt in chunks:
                                src = tt[:, (wb + a) % SUP:(wb + a) % SUP + (b2 - a)]
                                nc.scalar.activation(
                                    qp[:, a:b2], src,
                                    AF.Square, bias=-16.0, scale=1.0)
                    # poly pass (all-SBUF f16):
                    #   DVE region [0,kb):  ll = (a-8)*a = t^2-16  (exp bias -48)
                    #   Pool region [kb,B): ll = (a-4)^2 = t^2     (exp bias -64)
                    kb = B - B_POOL_SUPS * SUP
                    nc.vector.scalar_tensor_tensor(
                        ll_all[:, 0:kb], u_all[:, 0:kb], 8.0, u_all[:, 0:kb],
                        OP.subtract, OP.mult)
                    hh = llp.tile([128, B_POOL_SUPS * SUP], f16,
                                  name=f"hh{rep}_{mb}", tag="hh")
                    nc.gpsimd.tensor_scalar(out=hh[:], in0=u_all[:, kb:B],
                                            scalar1=-4.0, scalar2=None,
                                            op0=OP.add)
                    nc.gpsimd.tensor_tensor(out=ll_all[:, kb:B], in0=hh[:],
                                            in1=hh[:], op=OP.mult)
                    # ---- window tail (SBUF only) ----
                    eew = wpool.tile([128, W], bf16, name="eew")
                    nc.scalar.activation(eew[:], ll_all[:, wb:we], AF.Exp,
                                         bias=-XSH, scale=1.0)
                    scrw = wpool.tile([128, W], bf16, name="scrw")
                    nc.vector.scalar_tensor_tensor(
                        scrw[:], eew[:], 1.0, negw_sb[:, mb * W:(mb + 1) * W],
                        OP.mult, OP.mult, accum_out=scol[:, 3:4])
                    mq = wpool.tile([128, W], f32, name="mq")
                    nc.vector.scalar_tensor_tensor(
                        mq[:], qp[:], CP, posw_sb[:, mb * W:(mb + 1) * W],
                        OP.add, OP.mult)
                    nc.vector.reduce_max(mt_all[:, mb:mb + 1], mq[:], axis=AX.X)
                    nmt = vpool.tile([128, 1], f32, name="nmt")
                    nc.vector.tensor_scalar(out=nmt[:], in0=mt_all[:, mb:mb + 1],
                                            scalar1=-1.0, scalar2=None,
                                            op0=OP.mult)
                    epw = wpool.tile([128, W], f32, name="epw")
                    nc.scalar.activation(epw[:], mq[:], AF.Exp, bias=nmt[:],
                                         scale=1.0,
                                         accum_out=sp_all[:, mb:mb + 1])
                    # ---- bulk masked exp: big segments skipping window ----
                    # [we,kb) and [0,wb) use bias -48; Pool region [kb,B) -64
                    nc.scalar.activation(ee_all[:, we:kb], ll_all[:, we:kb],
                                         AF.Exp, bias=-XSH, scale=1.0,
                                         accum_out=scol[:, 0:1])
                    nc.scalar.activation(ee_all[:, kb:B], ll_all[:, kb:B],
                                         AF.Exp, bias=-(XSH + 16.0), scale=1.0,
                                         accum_out=scol[:, 1:2])
                    if wb > 0:
                        nc.scalar.activation(ee_all[:, 0:wb], ll_all[:, 0:wb],
                                             AF.Exp, bias=-XSH, scale=1.0,
                                             accum_out=scol[:, 2:3])
                        nc.vector.reduce_sum(sn_all[:, mb:mb + 1],
                                             scol[:, 0:4], axis=AX.X)
                    else:
                        # scol[:,2] unwritten: sum cols {0,1,3}
                        s01 = vpool.tile([128, 1], f32, name="s01")
                        nc.vector.tensor_tensor(out=s01[:], in0=scol[:, 0:1],
                                                in1=scol[:, 1:2], op=OP.add)
                        nc.vector.tensor_tensor(out=sn_all[:, mb:mb + 1],
                                                in0=s01[:], in1=scol[:, 3:4],
                                                op=OP.add)
                nc.sync.dma_start(OUT[:, 0:NMB], sn_all[:])
                nc.sync.dma_start(OUT[:, NMB:2 * NMB], sp_all[:])
                nc.sync.dma_start(OUT[:, 2 * NMB:3 * NMB], mt_all[:])

    nc.compile()
    return nc


def _host_prep(feat, label):
    feat = np.asarray(feat, dtype=np.float32)
    label = np.asarray(label).astype(np.int64).ravel()
    order = np.argsort(label, kind='stable')
    labs = label[order]
    f = feat[order]
    nrm = np.maximum(np.sqrt((f * f).sum(1, dtype=np.float32)), 1e-12).astype(np.float32)
    g = (f * (4.0 / nrm)[:, None]).astype(np.float32)
    gT = np.ascontiguousarray(g.T)  # [128, B]

    cnt = np.bincount(labs, minlength=512)
    cnt_row = cnt[labs]                       # class size per sorted row
    valid = (cnt_row >= 2) & (B - cnt_row >= 1)
    n_valid = max(int(valid.sum()), 1)

    negeye = (-16.0 * np.eye(128, dtype=np.float32))
    ident = np.eye(128, dtype=np.float32)

    import ml_dtypes
    in_maps, ok = [], True
    for c in range(M):
        shift = ROT - c * RB
        gT_c = np.roll(gT, shift, axis=1)
        labs_c = np.roll(labs, shift)
        posw = np.zeros((RB, W), dtype=np.float32)
        negw = np.zeros((RB, W), dtype=np.float32)
        for mb in range(NMB):
            rl = labs_c[ROT + mb * 128: ROT + (mb + 1) * 128]       # row labels
            wl = labs_c[mb * 128: mb * 128 + W]                     # window labels
            same = (rl[:, None] == wl[None, :])
            # coverage: window must contain every same-class column
            if not (same.sum(1) == cnt[rl]).all():
                ok = False
            pos = same.copy()
            pos[np.arange(128), np.arange(128) + ROT] = False       # drop diagonal
            r0 = mb * 128
            posw[r0:r0 + 128] = pos
            negw[r0:r0 + 128] = ~same
        in_maps.append({
            "gt": gT_c.astype(np.float32),
            "posw": posw.astype(ml_dtypes.bfloat16),
            "negw": negw.astype(ml_dtypes.bfloat16),
            "negeye": negeye,
            "ident": ident,
        })
    return in_maps, n_valid, ok


def _numpy_fallback(feat, label):
    feat = np.asarray(feat, dtype=np.float32)
    label = np.asarray(label).astype(np.int64).ravel()
    nrm = np.maximum(np.linalg.norm(feat, axis=1, keepdims=True), 1e-12)
    f = feat / nrm
    sim = (f @ f.T).astype(np.float32)
    same = label[:, None] == label[None, :]
    eye = np.eye(B, dtype=bool)
    pos_m, neg_m = same & ~eye, ~same
    ap = np.maximum(1.25 - sim, 0)
    an = np.maximum(sim + 0.25, 0)
    lp = np.where(pos_m, -256.0 * ap * (sim - 0.75), -1e9)
    ln_ = np.where(neg_m, 256.0 * an * (sim - 0.25), -1e9)

    def lse(x):
        m = x.max(1, keepdims=True)
        return (m + np.log(np.exp(x - m).sum(1, keepdims=True)))[:, 0]
    val = pos_m.any(1) & neg_m.any(1)
    x = lse(lp) + lse(ln_)
    loss = np.maximum(x, 0) + np.log1p(np.exp(-np.abs(x)))
    return np.float32(np.where(val, loss, 0.0).sum() / max(int(val.sum()), 1))


def _host_finish(res, n_valid):
    total = 0.0
    for c in range(M):
        o = np.asarray(res.results[c]["oloss"], dtype=np.float64)
        sn, sp, mt = o[:, 0:NMB], o[:, NMB:2 * NMB], o[:, 2 * NMB:3 * NMB]
        x = np.log(sp) + np.log(sn) + mt + (LSE_N_SH - CP - 16.0)
        total += np.logaddexp(0.0, x).sum()
    return np.float32(total / n_valid)


def kernel(feat, label):
    from concourse import bass_utils
    in_maps, n_valid, ok = _host_prep(feat, label)
    if not ok:
        return _numpy_fallback(feat, label)
    if 'nc' not in _CACHE:
        _CACHE['nc'] = _build(repeat=1)
    res = bass_utils.run_bass_kernel_spmd(_CACHE['nc'], in_maps,
                                          core_ids=list(range(M)))
    _CACHE['last'] = res
    return _host_finish(res, n_valid)


# revision 23
# speedup vs baseline: 472.8291x; 2.6912x over previous
"""CircleLoss (B=8192, D=128, 512 classes) on 8 Trainium2 NeuronCores.

Strategy (per sharding hint): rows of the sim matrix are data-parallel over
8 cores (1024 rows each); normalized features are replicated. Each core
computes its 1024 x 8192 slab of t = 16*sim via fp32r matmuls and reduces
per-row partial sums (sn, sp, mt); the host finishes with ln/softplus and
the scalar reduction across cores.

Math: with t = 16*sim,
  logit_n = 256*relu(s+.25)*(s-.25) = t^2-16 for t>=-4, else 0.
  Device computes ll = max(t,-1)*t (== t^2 for t>=-1; for t<-1 the value
  exp(ll-64) <= e^-48 is negligible exactly like the true branch), then
  sn = sum_j exp(ll-64) over negatives => lse_n = ln(sn) + 48.
  logit_p = (t-16)^2 - 16 exactly (s <= 1); masked positives use the
  qp+2048 shift so invalid rows vanish through softplus on the host.
Rows are sorted by label and columns rotated per-core so each row-block's
positives live in a static 384-wide window inside bulk supertiles 0/1 and
the sim diagonal sits in a static 128x128 block (removed by accumulating
-16*I into PSUM).  The bulk masked exp runs as two big segments per
row-block that skip the window, so only Exp/Square tables are used (no
activation-table swaps), and the poly pass is a single dual-PSUM STT
split between the Vector and GpSimd engines.
"""
import sys, os
sys.path.insert(0, '/opt/trn_rl_repo')
import numpy as np

B, D, M = 8192, 128, 8
RB = B // M          # rows per core
NMB = RB // 128      # row blocks per core
W = 384              # positives window width
SUP = 1024           # bulk column supertile
NSUP = B // SUP
ROT = 128            # rotation pad: core's rows sit at local cols [128,1152)
CP = 2048.0          # p-branch mask shift
XSH = 48.0           # bulk exp shift: exp(ll - 48), ll = (a-8)*a = logit_n
LSE_N_SH = 48.0      # host adds back: lse_n = ln(sn) + 48

# engine schedule knobs: clamp-pass supertiles done on ACT (rest on DVE)
def A_ACT_SBS(mb):
    return {3, 6} if mb % 2 == 0 else {3}

_CACHE = {}


def _build(repeat=1):
    import concourse.bass as bass
    import concourse.tile as tile
    from concourse import bacc, mybir

    f32, f16, bf16 = mybir.dt.float32, mybir.dt.float16, mybir.dt.bfloat16
    f32r = mybir.dt.float32r
    AF = mybir.ActivationFunctionType
    OP = mybir.AluOpType
    AX = mybir.AxisListType

    nc = bacc.Bacc("TRN2", target_bir_lowering=False, debug=False, num_devices=M)

    def reg_const(value, dtype=f32):
        t = nc.alloc_sbuf_tensor(f"constx-{dtype.name}-{value}", [128, 1], dtype)
        nc.gpsimd.memset(t.ap(), value)
        nc.const_aps.aps[(dtype, value)] = t.ap()
    for v in (4.0, -16.0, -XSH, -(XSH + 16.0)):
        reg_const(v)

    GT = nc.dram_tensor("gt", [128, B], f32, kind="ExternalInput").ap()
    POSW = nc.dram_tensor("posw", [RB, W], bf16, kind="ExternalInput").ap()
    NEGW = nc.dram_tensor("negw", [RB, W], bf16, kind="ExternalInput").ap()
    NEGEYE = nc.dram_tensor("negeye", [128, 128], f32, kind="ExternalInput").ap()
    IDENT = nc.dram_tensor("ident", [128, 128], f32, kind="ExternalInput").ap()
    OUT = nc.dram_tensor("oloss", [128, 3 * NMB], f32, kind="ExternalOutput").ap()

    with tile.TileContext(nc) as tc:
        with (
            tc.tile_pool(name="gp", bufs=1) as gp,
            tc.tile_pool(name="mp", bufs=1) as mp,
            tc.tile_pool(name="tsupp", bufs=3, space="PSUM") as tsupp,
            tc.tile_pool(name="llp", bufs=2) as llp,
            tc.tile_pool(name="eep", bufs=2) as eep,
            tc.tile_pool(name="wp", bufs=2) as wpool,
            tc.tile_pool(name="vp", bufs=3) as vpool,
            tc.tile_pool(name="scp", bufs=2) as scp,
            tc.tile_pool(name="fin", bufs=1) as fin,
        ):
            gt_sb = gp.tile([128, B], f32r)
            for c in range(NSUP):
                nc.sync.dma_start(gt_sb[:, c * SUP:(c + 1) * SUP],
                                  GT[:, c * SUP:(c + 1) * SUP].bitcast(f32r))
            ne_sb = gp.tile([128, 128], f32r)
            nc.sync.dma_start(ne_sb[:], NEGEYE[:].bitcast(f32r))
            id_sb = gp.tile([128, 128], f32r)
            nc.sync.dma_start(id_sb[:], IDENT[:].bitcast(f32r))
            posw_sb = mp.tile([128, NMB * W], bf16)
            negw_sb = mp.tile([128, NMB * W], bf16)
            for mb in range(NMB):
                nc.sync.dma_start(posw_sb[:, mb * W:(mb + 1) * W],
                                  POSW[mb * 128:(mb + 1) * 128, :])
                nc.sync.dma_start(negw_sb[:, mb * W:(mb + 1) * W],
                                  NEGW[mb * 128:(mb + 1) * 128, :])

            for rep in range(repeat):
                sn_all = fin.tile([128, NMB], f32, name=f"sn{rep}", tag="sn")
                sp_all = fin.tile([128, NMB], f32, name=f"sp{rep}", tag="sp")
                mt_all = fin.tile([128, NMB], f32, name=f"mt{rep}", tag="mt")
                for mb in range(NMB):
                    lhs = gt_sb[:, ROT + mb * 128: ROT + (mb + 1) * 128]
                    wb, we = mb * 128, mb * 128 + W     # window cols (global)
                    dcol = ROT + mb * 128               # diagonal block col
                    u_all = llp.tile([128, B], f16, name=f"u{rep}_{mb}",
                                     tag="u")
                    ll_all = llp.tile([128, B], f16, name=f"ll{rep}_{mb}",
                                      tag="ll")
                    ee_all = eep.tile([128, B], bf16, name=f"ee{rep}_{mb}",
                                      tag="ee")
                    scol = scp.tile([128, 4], f32, name=f"scol{rep}_{mb}",
                                    tag="scol")
                    qp = wpool.tile([128, W], f32, name="qp")
                    tsup = []
                    for sb in range(NSUP):
                        t = tsupp.tile([128, SUP], f32, name="tsup")
                        tsup.append(t)
                        for h in range(SUP // 512):
                            nc.tensor.matmul(
                                t[:, h * 512:(h + 1) * 512], lhs,
                                gt_sb[:, sb * SUP + h * 512: sb * SUP + (h + 1) * 512],
                                start=True, stop=True)
                        if dcol // SUP == sb:
                            off = dcol % SUP
                            nc.tensor.matmul(t[:, off:off + 128], ne_sb[:], id_sb[:],
                                             start=False, stop=True,
                                             skip_group_check=True)
                        # clamp pass: a = relu(t+4)  (PSUM f32 -> SBUF f16)
                        useg = u_all[:, sb * SUP:(sb + 1) * SUP]
                        if sb in A_ACT_SBS(mb):
                            nc.scalar.activation(useg, t[:], AF.Relu,
                                                 bias=4.0, scale=1.0)
                        else:
                            nc.vector.tensor_scalar(
                                out=useg, in0=t[:], scalar1=4.0, scalar2=0.0,
                                op0=OP.add, op1=OP.max)
                        if sb == 1:
                            # positives need exact (t-16)^2 from PSUM sup 0/1
                            ca = min(SUP - wb, W)       # chunk split point
                            chunks = [(0, ca, tsup[0])]
                            if ca < W:
                                chunks.append((ca, W, tsup[1]))
                            for a, b2, tt in chunks:
                                src = tt[:, (wb + a) % SUP:(wb + a) % SUP + (b2 - a)]
                                nc.scalar.activation(
                                    qp[:, a:b2], src,
                                    AF.Square, bias=-16.0, scale=1.0)
                    # poly pass: ll = (a-8)*a = t^2-16 (one full-tile f16 STT,
                    # eligible for the DVE merged-AP fast path)
                    nc.vector.scalar_tensor_tensor(
                        ll_all[:], u_all[:], 8.0, u_all[:],
                        OP.subtract, OP.mult)
                    # ---- window tail (SBUF only) ----
                    eew = wpool.tile([128, W], bf16, name="eew")
                    nc.scalar.activation(eew[:], ll_all[:, wb:we], AF.Exp,
                                         bias=-XSH, scale=1.0)
                    scrw = wpool.tile([128, W], bf16, name="scrw")
                    nc.vector.scalar_tensor_tensor(
                        scrw[:], eew[:], 1.0, negw_sb[:, mb * W:(mb + 1) * W],
                        OP.mult, OP.mult, accum_out=scol[:, 2:3])
                    mq = wpool.tile([128, W], f32, name="mq")
                    nc.vector.scalar_tensor_tensor(
                        mq[:], qp[:], CP, posw_sb[:, mb * W:(mb + 1) * W],
                        OP.add, OP.mult)
                    nc.vector.reduce_max(mt_all[:, mb:mb + 1], mq[:], axis=AX.X)
                    nmt = vpool.tile([128, 1], f32, name="nmt")
                    nc.vector.tensor_scalar(out=nmt[:], in0=mt_all[:, mb:mb + 1],
                                            scalar1=-1.0, scalar2=None,
                                            op0=OP.mult)
                    epw = wpool.tile([128, W], f32, name="epw")
                    nc.scalar.activation(epw[:], mq[:], AF.Exp, bias=nmt[:],
                                         scale=1.0,
                                         accum_out=sp_all[:, mb:mb + 1])
                    # ---- bulk masked exp: big segments skipping window ----
                    nc.scalar.activation(ee_all[:, we:B], ll_all[:, we:B],
                                         AF.Exp, bias=-XSH, scale=1.0,
                                         accum_out=scol[:, 0:1])
                    if wb > 0:
                        nc.scalar.activation(ee_all[:, 0:wb], ll_all[:, 0:wb],
                                             AF.Exp, bias=-XSH, scale=1.0,
                                             accum_out=scol[:, 1:2])
                        nc.vector.reduce_sum(sn_all[:, mb:mb + 1],
                                             scol[:, 0:3], axis=AX.X)
                    else:
                        # scol[:,1] unwritten: sum cols {0,2}
                        nc.vector.tensor_tensor(out=sn_all[:, mb:mb + 1],
                                                in0=scol[:, 0:1],
                                                in1=scol[:, 2:3], op=OP.add)
                nc.sync.dma_start(OUT[:, 0:NMB], sn_all[:])
                nc.sync.dma_start(OUT[:, NMB:2 * NMB], sp_all[:])
                nc.sync.dma_start(OUT[:, 2 * NMB:3 * NMB], mt_all[:])

    nc.compile()
    return nc


def _host_prep(feat, label):
    feat = np.asarray(feat, dtype=np.float32)
    label = np.asarray(label).astype(np.int64).ravel()
    order = np.argsort(label, kind='stable')
    labs = label[order]
    f = feat[order]
    nrm = np.maximum(np.sqrt((f * f).sum(1, dtype=np.float32)), 1e-12).astype(np.float32)
    g = (f * (4.0 / nrm)[:, None]).astype(np.float32)
    gT = np.ascontiguousarray(g.T)  # [128, B]

    cnt = np.bincount(labs, minlength=512)
    cnt_row = cnt[labs]                       # class size per sorted row
    valid = (cnt_row >= 2) & (B - cnt_row >= 1)
    n_valid = max(int(valid.sum()), 1)

    negeye = (-16.0 * np.eye(128, dtype=np.float32))
    ident = np.eye(128, dtype=np.float32)

    import ml_dtypes
    in_maps, ok = [], True
    for c in range(M):
        shift = ROT - c * RB
        gT_c = np.roll(gT, shift, axis=1)
        labs_c = np.roll(labs, shift)
        posw = np.zeros((RB, W), dtype=np.float32)
        negw = np.zeros((RB, W), dtype=np.float32)
        for mb in range(NMB):
            rl = labs_c[ROT + mb * 128: ROT + (mb + 1) * 128]       # row labels
            wl = labs_c[mb * 128: mb * 128 + W]                     # window labels
            same = (rl[:, None] == wl[None, :])
            # coverage: window must contain every same-class column
            if not (same.sum(1) == cnt[rl]).all():
                ok = False
            pos = same.copy()
            pos[np.arange(128), np.arange(128) + ROT] = False       # drop diagonal
            r0 = mb * 128
            posw[r0:r0 + 128] = pos
            negw[r0:r0 + 128] = ~same
        in_maps.append({
            "gt": gT_c.astype(np.float32),
            "posw": posw.astype(ml_dtypes.bfloat16),
            "negw": negw.astype(ml_dtypes.bfloat16),
            "negeye": negeye,
            "ident": ident,
        })
    return in_maps, n_valid, ok


def _numpy_fallback(feat, label):
    feat = np.asarray(feat, dtype=np.float32)
    label = np.asarray(label).astype(np.int64).ravel()
    nrm = np.maximum(np.linalg.norm(feat, axis=1, keepdims=True), 1e-12)
    f = feat / nrm
    sim = (f @ f.T).astype(np.float32)
    same = label[:, None] == label[None, :]
    eye = np.eye(B, dtype=bool)
    pos_m, neg_m = same & ~eye, ~same
    ap = np.maximum(1.25 - sim, 0)
    an = np.maximum(sim + 0.25, 0)
    lp = np.where(pos_m, -256.0 * ap * (sim - 0.75), -1e9)
    ln_ = np.where(neg_m, 256.0 * an * (sim - 0.25), -1e9)

    def lse(x):
        m = x.max(1, keepdims=True)
        return (m + np.log(np.exp(x - m).sum(1, keepdims=True)))[:, 0]
    val = pos_m.any(1) & neg_m.any(1)
    x = lse(lp) + lse(ln_)
    loss = np.maximum(x, 0) + np.log1p(np.exp(-np.abs(x)))
    return np.float32(np.where(val, loss, 0.0).sum() / max(int(val.sum()), 1))


def _host_finish(res, n_valid):
    total = 0.0
    for c in range(M):
        o = np.asarray(res.results[c]["oloss"], dtype=np.float64)
        sn, sp, mt = o[:, 0:NMB], o[:, NMB:2 * NMB], o[:, 2 * NMB:3 * NMB]
        x = np.log(sp) + np.log(sn) + mt + (LSE_N_SH - CP - 16.0)
        total += np.logaddexp(0.0, x).sum()
    return np.float32(total / n_valid)


def kernel(feat, label):
    from concourse import bass_utils
    in_maps, n_valid, ok = _host_prep(feat, label)
    if not ok:
        return _numpy_fallback(feat, label)
    if 'nc' not in _CACHE:
        _CACHE['nc'] = _build(repeat=1)
    res = bass_utils.run_bass_kernel_spmd(_CACHE['nc'], in_maps,
                                          core_ids=list(range(M)))
    _CACHE['last'] = res
    return _host_finish(res, n_valid)


# revision 25
# speedup vs baseline: 529.6797x; 1.1202x over previous
"""CircleLoss (B=8192, D=128, 512 classes) on 8 Trainium2 NeuronCores.

Strategy (per sharding hint): rows of the sim matrix are data-parallel over
8 cores (1024 rows each); normalized features are replicated. Each core
computes its 1024 x 8192 slab of t = 16*sim via fp32r matmuls and reduces
per-row partial sums (sn, sp, mt); the host finishes with ln/softplus and
the scalar reduction across cores.

Math: with t = 16*sim,
  logit_n = 256*relu(s+.25)*(s-.25) = t^2-16 for t>=-4, else 0.
  Device computes ll = max(t,-1)*t (== t^2 for t>=-1; for t<-1 the value
  exp(ll-64) <= e^-48 is negligible exactly like the true branch), then
  sn = sum_j exp(ll-64) over negatives => lse_n = ln(sn) + 48.
  logit_p = (t-16)^2 - 16 exactly (s <= 1); masked positives use the
  qp+2048 shift so invalid rows vanish through softplus on the host.
Rows are sorted by label and columns rotated per-core so each row-block's
positives live in a static 384-wide window inside bulk supertiles 0/1 and
the sim diagonal sits in a static 128x128 block (removed by accumulating
-16*I into PSUM).  The bulk masked exp runs as two big segments per
row-block that skip the window, so only Exp/Square tables are used (no
activation-table swaps), and the poly pass is a single dual-PSUM STT
split between the Vector and GpSimd engines.
"""
import sys, os
sys.path.insert(0, '/opt/trn_rl_repo')
import numpy as np

B, D, M = 8192, 128, 8
RB = B // M          # rows per core
NMB = RB // 128      # row blocks per core
W = 384              # positives window width
SUP = 1024           # bulk column supertile
NSUP = B // SUP
ROT = 128            # rotation pad: core's rows sit at local cols [128,1152)
CP = 2048.0          # p-branch mask shift
XSH = 64.0           # bulk exp shift: exp(h*h - 64) = exp(logit_n - 48)
LSE_N_SH = 48.0      # host adds back: lse_n = ln(sn) + 48
KS = 2048            # poly split: [0,KS) DVE STT, [KS,B) ACT Square

_CACHE = {}


def _build(repeat=1):
    import concourse.bass as bass
    import concourse.tile as tile
    from concourse import bacc, mybir

    f32, f16, bf16 = mybir.dt.float32, mybir.dt.float16, mybir.dt.bfloat16
    f32r = mybir.dt.float32r
    AF = mybir.ActivationFunctionType
    OP = mybir.AluOpType
    AX = mybir.AxisListType

    nc = bacc.Bacc("TRN2", target_bir_lowering=False, debug=False, num_devices=M)

    def reg_const(value, dtype=f32):
        t = nc.alloc_sbuf_tensor(f"constx-{dtype.name}-{value}", [128, 1], dtype)
        nc.gpsimd.memset(t.ap(), value)
        nc.const_aps.aps[(dtype, value)] = t.ap()
    for v in (-16.0, -XSH):
        reg_const(v)

    GT = nc.dram_tensor("gt", [128, B], f32, kind="ExternalInput").ap()
    POSW = nc.dram_tensor("posw", [RB, W], bf16, kind="ExternalInput").ap()
    NEGW = nc.dram_tensor("negw", [RB, W], bf16, kind="ExternalInput").ap()
    NEGEYE = nc.dram_tensor("negeye", [128, 128], f32, kind="ExternalInput").ap()
    IDENT = nc.dram_tensor("ident", [128, 128], f32, kind="ExternalInput").ap()
    OUT = nc.dram_tensor("oloss", [128, 3 * NMB], f32, kind="ExternalOutput").ap()

    with tile.TileContext(nc) as tc:
        with (
            tc.tile_pool(name="gp", bufs=1) as gp,
            tc.tile_pool(name="mp", bufs=1) as mp,
            tc.tile_pool(name="tsupp", bufs=3, space="PSUM") as tsupp,
            tc.tile_pool(name="llp", bufs=2) as llp,
            tc.tile_pool(name="eep", bufs=2) as eep,
            tc.tile_pool(name="wp", bufs=2) as wpool,
            tc.tile_pool(name="vp", bufs=3) as vpool,
            tc.tile_pool(name="scp", bufs=2) as scp,
            tc.tile_pool(name="fin", bufs=1) as fin,
        ):
            gt_sb = gp.tile([128, B], f32r)
            for c in range(NSUP):
                nc.sync.dma_start(gt_sb[:, c * SUP:(c + 1) * SUP],
                                  GT[:, c * SUP:(c + 1) * SUP].bitcast(f32r))
            ne_sb = gp.tile([128, 128], f32r)
            nc.sync.dma_start(ne_sb[:], NEGEYE[:].bitcast(f32r))
            id_sb = gp.tile([128, 128], f32r)
            nc.sync.dma_start(id_sb[:], IDENT[:].bitcast(f32r))
            posw_sb = mp.tile([128, NMB * W], bf16)
            negw_sb = mp.tile([128, NMB * W], bf16)
            for mb in range(NMB):
                nc.sync.dma_start(posw_sb[:, mb * W:(mb + 1) * W],
                                  POSW[mb * 128:(mb + 1) * 128, :])
                nc.sync.dma_start(negw_sb[:, mb * W:(mb + 1) * W],
                                  NEGW[mb * 128:(mb + 1) * 128, :])

            for rep in range(repeat):
                sn_all = fin.tile([128, NMB], f32, name=f"sn{rep}", tag="sn")
                sp_all = fin.tile([128, NMB], f32, name=f"sp{rep}", tag="sp")
                mt_all = fin.tile([128, NMB], f32, name=f"mt{rep}", tag="mt")
                for mb in range(NMB):
                    lhs = gt_sb[:, ROT + mb * 128: ROT + (mb + 1) * 128]
                    wb, we = mb * 128, mb * 128 + W     # window cols (global)
                    dcol = ROT + mb * 128               # diagonal block col
                    u_all = llp.tile([128, B], f16, name=f"u{rep}_{mb}",
                                     tag="u")
                    ll_all = llp.tile([128, B], f16, name=f"ll{rep}_{mb}",
                                      tag="ll")
                    ee_all = eep.tile([128, B], bf16, name=f"ee{rep}_{mb}",
                                      tag="ee")
                    scol = scp.tile([128, 4], f32, name=f"scol{rep}_{mb}",
                                    tag="scol")
                    qp = wpool.tile([128, W], f32, name="qp")
                    tsup = []
                    for sb in range(NSUP):
                        t = tsupp.tile([128, SUP], f32, name="tsup")
                        tsup.append(t)
                        for h in range(SUP // 512):
                            nc.tensor.matmul(
                                t[:, h * 512:(h + 1) * 512], lhs,
                                gt_sb[:, sb * SUP + h * 512: sb * SUP + (h + 1) * 512],
                                start=True, stop=True)
                        if dcol // SUP == sb:
                            off = dcol % SUP
                            nc.tensor.matmul(t[:, off:off + 128], ne_sb[:], id_sb[:],
                                             start=False, stop=True,
                                             skip_group_check=True)
                        # clamp pass: h = max(t,-4)  (PSUM f32 -> SBUF f16)
                        useg = u_all[:, sb * SUP:(sb + 1) * SUP]
                        nc.vector.tensor_scalar(
                            out=useg, in0=t[:], scalar1=-4.0, scalar2=None,
                            op0=OP.max)
                        if sb == 1:
                            # positives need exact (t-16)^2 from PSUM sup 0/1
                            ca = min(SUP - wb, W)       # chunk split point
                            chunks = [(0, ca, tsup[0])]
                            if ca < W:
                                chunks.append((ca, W, tsup[1]))
                            for a, b2, tt in chunks:
                                src = tt[:, (wb + a) % SUP:(wb + a) % SUP + (b2 - a)]
                                nc.scalar.activation(
                                    qp[:, a:b2], src,
                                    AF.Square, bias=-16.0, scale=1.0)
                    # poly pass: ll = h*h = clamped t^2 (DVE head, ACT tail)
                    nc.vector.scalar_tensor_tensor(
                        ll_all[:, 0:KS], u_all[:, 0:KS], 1.0, u_all[:, 0:KS],
                        OP.mult, OP.mult)
                    nc.scalar.activation(ll_all[:, KS:B], u_all[:, KS:B],
                                         AF.Square, bias=0.0, scale=1.0)
                    # ---- window tail (SBUF only) ----
                    eew = wpool.tile([128, W], bf16, name="eew")
                    nc.scalar.activation(eew[:], ll_all[:, wb:we], AF.Exp,
                                         bias=-XSH, scale=1.0)
                    scrw = wpool.tile([128, W], bf16, name="scrw")
                    nc.vector.scalar_tensor_tensor(
                        scrw[:], eew[:], 1.0, negw_sb[:, mb * W:(mb + 1) * W],
                        OP.mult, OP.mult, accum_out=scol[:, 2:3])
                    mq = wpool.tile([128, W], f32, name="mq")
                    nc.vector.scalar_tensor_tensor(
                        mq[:], qp[:], CP, posw_sb[:, mb * W:(mb + 1) * W],
                        OP.add, OP.mult)
                    nc.vector.reduce_max(mt_all[:, mb:mb + 1], mq[:], axis=AX.X)
                    nmt = vpool.tile([128, 1], f32, name="nmt")
                    nc.vector.tensor_scalar(out=nmt[:], in0=mt_all[:, mb:mb + 1],
                                            scalar1=-1.0, scalar2=None,
                                            op0=OP.mult)
                    epw = wpool.tile([128, W], f32, name="epw")
                    nc.scalar.activation(epw[:], mq[:], AF.Exp, bias=nmt[:],
                                         scale=1.0,
                                         accum_out=sp_all[:, mb:mb + 1])
                    # ---- bulk masked exp: big segments skipping window ----
                    nc.scalar.activation(ee_all[:, we:B], ll_all[:, we:B],
                                         AF.Exp, bias=-XSH, scale=1.0,
                                         accum_out=scol[:, 0:1])
                    if wb > 0:
                        nc.scalar.activation(ee_all[:, 0:wb], ll_all[:, 0:wb],
                                             AF.Exp, bias=-XSH, scale=1.0,
                                             accum_out=scol[:, 1:2])
                        nc.vector.reduce_sum(sn_all[:, mb:mb + 1],
                                             scol[:, 0:3], axis=AX.X)
                    else:
                        # scol[:,1] unwritten: sum cols {0,2}
                        nc.vector.tensor_tensor(out=sn_all[:, mb:mb + 1],
                                                in0=scol[:, 0:1],
                                                in1=scol[:, 2:3], op=OP.add)
                nc.sync.dma_start(OUT[:, 0:NMB], sn_all[:])
                nc.sync.dma_start(OUT[:, NMB:2 * NMB], sp_all[:])
                nc.sync.dma_start(OUT[:, 2 * NMB:3 * NMB], mt_all[:])

    nc.compile()
    return nc


def _host_prep(feat, label):
    feat = np.asarray(feat, dtype=np.float32)
    label = np.asarray(label).astype(np.int64).ravel()
    order = np.argsort(label, kind='stable')
    labs = label[order]
    f = feat[order]
    nrm = np.maximum(np.sqrt((f * f).sum(1, dtype=np.float32)), 1e-12).astype(np.float32)
    g = (f * (4.0 / nrm)[:, None]).astype(np.float32)
    gT = np.ascontiguousarray(g.T)  # [128, B]

    cnt = np.bincount(labs, minlength=512)
    cnt_row = cnt[labs]                       # class size per sorted row
    valid = (cnt_row >= 2) & (B - cnt_row >= 1)
    n_valid = max(int(valid.sum()), 1)

    negeye = (-16.0 * np.eye(128, dtype=np.float32))
    ident = np.eye(128, dtype=np.float32)

    import ml_dtypes
    in_maps, ok = [], True
    for c in range(M):
        shift = ROT - c * RB
        gT_c = np.roll(gT, shift, axis=1)
        labs_c = np.roll(labs, shift)
        posw = np.zeros((RB, W), dtype=np.float32)
        negw = np.zeros((RB, W), dtype=np.float32)
        for mb in range(NMB):
            rl = labs_c[ROT + mb * 128: ROT + (mb + 1) * 128]       # row labels
            wl = labs_c[mb * 128: mb * 128 + W]                     # window labels
            same = (rl[:, None] == wl[None, :])
            # coverage: window must contain every same-class column
            if not (same.sum(1) == cnt[rl]).all():
                ok = False
            pos = same.copy()
            pos[np.arange(128), np.arange(128) + ROT] = False       # drop diagonal
            r0 = mb * 128
            posw[r0:r0 + 128] = pos
            negw[r0:r0 + 128] = ~same
        in_maps.append({
            "gt": gT_c.astype(np.float32),
            "posw": posw.astype(ml_dtypes.bfloat16),
            "negw": negw.astype(ml_dtypes.bfloat16),
            "negeye": negeye,
            "ident": ident,
        })
    return in_maps, n_valid, ok


def _numpy_fallback(feat, label):
    feat = np.asarray(feat, dtype=np.float32)
    label = np.asarray(label).astype(np.int64).ravel()
    nrm = np.maximum(np.linalg.norm(feat, axis=1, keepdims=True), 1e-12)
    f = feat / nrm
    sim = (f @ f.T).astype(np.float32)
    same = label[:, None] == label[None, :]
    eye = np.eye(B, dtype=bool)
    pos_m, neg_m = same & ~eye, ~same
    ap = np.maximum(1.25 - sim, 0)
    an = np.maximum(sim + 0.25, 0)
    lp = np.where(pos_m, -256.0 * ap * (sim - 0.75), -1e9)
    ln_ = np.where(neg_m, 256.0 * an * (sim - 0.25), -1e9)

    def lse(x):
        m = x.max(1, keepdims=True)
        return (m + np.log(np.exp(x - m).sum(1, keepdims=True)))[:, 0]
    val = pos_m.any(1) & neg_m.any(1)
    x = lse(lp) + lse(ln_)
    loss = np.maximum(x, 0) + np.log1p(np.exp(-np.abs(x)))
    return np.float32(np.where(val, loss, 0.0).sum() / max(int(val.sum()), 1))


def _host_finish(res, n_valid):
    total = 0.0
    for c in range(M):
        o = np.asarray(res.results[c]["oloss"], dtype=np.float64)
        sn, sp, mt = o[:, 0:NMB], o[:, NMB:2 * NMB], o[:, 2 * NMB:3 * NMB]
        x = np.log(sp) + np.log(sn) + mt + (LSE_N_SH - CP - 16.0)
        total += np.logaddexp(0.0, x).sum()
    return np.float32(total / n_valid)


def kernel(feat, label):
    from concourse import bass_utils
    in_maps, n_valid, ok = _host_prep(feat, label)
    if not ok:
        return _numpy_fallback(feat, label)
    if 'nc' not in _CACHE:
        _CACHE['nc'] = _build(repeat=1)
    res = bass_utils.run_bass_kernel_spmd(_CACHE['nc'], in_maps,
                                          core_ids=list(range(M)))
    _CACHE['last'] = res
    return _host_finish(res, n_valid)


# revision 26
# speedup vs baseline: 674.2476x; 1.2729x over previous
"""CircleLoss (B=8192, D=128, 512 classes) on 8 Trainium2 NeuronCores.

Strategy (per sharding hint): rows of the sim matrix are data-parallel over
8 cores (1024 rows each); normalized features are replicated. Each core
computes its 1024 x 8192 slab of t = 16*sim via fp32r matmuls and reduces
per-row partial sums (sn, sp, mt); the host finishes with ln/softplus and
the scalar reduction across cores.

Math: with t = 16*sim,
  logit_n = 256*relu(s+.25)*(s-.25) = t^2-16 for t>=-4, else 0.
  Device computes ll = max(t,-1)*t (== t^2 for t>=-1; for t<-1 the value
  exp(ll-64) <= e^-48 is negligible exactly like the true branch), then
  sn = sum_j exp(ll-64) over negatives => lse_n = ln(sn) + 48.
  logit_p = (t-16)^2 - 16 exactly (s <= 1); masked positives use the
  qp+2048 shift so invalid rows vanish through softplus on the host.
Rows are sorted by label and columns rotated per-core so each row-block's
positives live in a static 384-wide window inside bulk supertiles 0/1 and
the sim diagonal sits in a static 128x128 block (removed by accumulating
-16*I into PSUM).  The bulk masked exp runs as two big segments per
row-block that skip the window, so only Exp/Square tables are used (no
activation-table swaps), and the poly pass is a single dual-PSUM STT
split between the Vector and GpSimd engines.
"""
import sys, os
sys.path.insert(0, '/opt/trn_rl_repo')
import numpy as np

B, D, M = 8192, 128, 8
RB = B // M          # rows per core
NMB = RB // 128      # row blocks per core
W = 320              # positives window width
SUP = 1024           # bulk column supertile
NSUP = B // SUP
ROT = 128            # rotation pad: core's rows sit at local cols [128,1152)
CP = 2048.0          # p-branch mask shift
XSH = 64.0           # bulk exp shift: exp(h*h - 64) = exp(logit_n - 48)
LSE_N_SH = 48.0      # host adds back: lse_n = ln(sn) + 48
KS = 3072            # poly split: [0,KS) DVE STT, [KS,B) ACT Square

_CACHE = {}


def _build(repeat=1):
    import concourse.bass as bass
    import concourse.tile as tile
    from concourse import bacc, mybir

    f32, f16, bf16 = mybir.dt.float32, mybir.dt.float16, mybir.dt.bfloat16
    f32r = mybir.dt.float32r
    AF = mybir.ActivationFunctionType
    OP = mybir.AluOpType
    AX = mybir.AxisListType

    nc = bacc.Bacc("TRN2", target_bir_lowering=False, debug=False, num_devices=M)

    def reg_const(value, dtype=f32):
        t = nc.alloc_sbuf_tensor(f"constx-{dtype.name}-{value}", [128, 1], dtype)
        nc.gpsimd.memset(t.ap(), value)
        nc.const_aps.aps[(dtype, value)] = t.ap()
    for v in (-16.0, -XSH):
        reg_const(v)

    GT = nc.dram_tensor("gt", [128, B], f32, kind="ExternalInput").ap()
    POSW = nc.dram_tensor("posw", [RB, W], bf16, kind="ExternalInput").ap()
    NEGW = nc.dram_tensor("negw", [RB, W], bf16, kind="ExternalInput").ap()
    NEGEYE = nc.dram_tensor("negeye", [128, 128], f32, kind="ExternalInput").ap()
    IDENT = nc.dram_tensor("ident", [128, 128], f32, kind="ExternalInput").ap()
    OUT = nc.dram_tensor("oloss", [128, 3 * NMB], f32, kind="ExternalOutput").ap()

    with tile.TileContext(nc) as tc:
        with (
            tc.tile_pool(name="gp", bufs=1) as gp,
            tc.tile_pool(name="mp", bufs=1) as mp,
            tc.tile_pool(name="tsupp", bufs=4, space="PSUM") as tsupp,
            tc.tile_pool(name="llp", bufs=2) as llp,
            tc.tile_pool(name="eep", bufs=2) as eep,
            tc.tile_pool(name="wp", bufs=2) as wpool,
            tc.tile_pool(name="vp", bufs=3) as vpool,
            tc.tile_pool(name="scp", bufs=2) as scp,
            tc.tile_pool(name="fin", bufs=1) as fin,
        ):
            gt_sb = gp.tile([128, B], f32r)
            for c in range(NSUP):
                nc.sync.dma_start(gt_sb[:, c * SUP:(c + 1) * SUP],
                                  GT[:, c * SUP:(c + 1) * SUP].bitcast(f32r))
            ne_sb = gp.tile([128, 128], f32r)
            nc.sync.dma_start(ne_sb[:], NEGEYE[:].bitcast(f32r))
            id_sb = gp.tile([128, 128], f32r)
            nc.sync.dma_start(id_sb[:], IDENT[:].bitcast(f32r))
            posw_sb = mp.tile([128, NMB * W], bf16)
            negw_sb = mp.tile([128, NMB * W], bf16)
            for mb in range(NMB):
                nc.sync.dma_start(posw_sb[:, mb * W:(mb + 1) * W],
                                  POSW[mb * 128:(mb + 1) * 128, :])
                nc.sync.dma_start(negw_sb[:, mb * W:(mb + 1) * W],
                                  NEGW[mb * 128:(mb + 1) * 128, :])

            for rep in range(repeat):
                sn_all = fin.tile([128, NMB], f32, name=f"sn{rep}", tag="sn")
                sp_all = fin.tile([128, NMB], f32, name=f"sp{rep}", tag="sp")
                mt_all = fin.tile([128, NMB], f32, name=f"mt{rep}", tag="mt")
                for mb in range(NMB):
                    lhs = gt_sb[:, ROT + mb * 128: ROT + (mb + 1) * 128]
                    wb, we = mb * 128, mb * 128 + W     # window cols (global)
                    dcol = ROT + mb * 128               # diagonal block col
                    u_all = llp.tile([128, B], f16, name=f"u{rep}_{mb}",
                                     tag="u")
                    ll_all = llp.tile([128, B], f16, name=f"ll{rep}_{mb}",
                                      tag="ll")
                    ee_all = eep.tile([128, B], bf16, name=f"ee{rep}_{mb}",
                                      tag="ee")
                    scol = scp.tile([128, 4], f32, name=f"scol{rep}_{mb}",
                                    tag="scol")
                    qp = wpool.tile([128, W], f32, name="qp")
                    tsup = []
                    for sb in range(NSUP):
                        t = tsupp.tile([128, SUP], f32, name="tsup")
                        tsup.append(t)
                        for h in range(SUP // 512):
                            nc.tensor.matmul(
                                t[:, h * 512:(h + 1) * 512], lhs,
                                gt_sb[:, sb * SUP + h * 512: sb * SUP + (h + 1) * 512],
                                start=True, stop=True)
                        if dcol // SUP == sb:
                            off = dcol % SUP
                            nc.tensor.matmul(t[:, off:off + 128], ne_sb[:], id_sb[:],
                                             start=False, stop=True,
                                             skip_group_check=True)
                        # clamp pass: h = max(t,-4)  (PSUM f32 -> SBUF f16)
                        useg = u_all[:, sb * SUP:(sb + 1) * SUP]
                        nc.vector.tensor_scalar(
                            out=useg, in0=t[:], scalar1=-4.0, scalar2=None,
                            op0=OP.max)
                        if sb == 1:
                            # positives need exact (t-16)^2 from PSUM sup 0/1
                            ca = min(SUP - wb, W)       # chunk split point
                            chunks = [(0, ca, tsup[0])]
                            if ca < W:
                                chunks.append((ca, W, tsup[1]))
                            for a, b2, tt in chunks:
                                src = tt[:, (wb + a) % SUP:(wb + a) % SUP + (b2 - a)]
                                nc.scalar.activation(
                                    qp[:, a:b2], src,
                                    AF.Square, bias=-16.0, scale=1.0)
                    # poly pass: ll = h*h = clamped t^2 (DVE head, ACT tail)
                    nc.vector.scalar_tensor_tensor(
                        ll_all[:, 0:KS], u_all[:, 0:KS], 1.0, u_all[:, 0:KS],
                        OP.mult, OP.mult)
                    nc.scalar.activation(ll_all[:, KS:B], u_all[:, KS:B],
                                         AF.Square, bias=0.0, scale=1.0)
                    # ---- window tail (SBUF only) ----
                    eew = wpool.tile([128, W], bf16, name="eew")
                    nc.scalar.activation(eew[:], ll_all[:, wb:we], AF.Exp,
                                         bias=-XSH, scale=1.0)
                    scrw = wpool.tile([128, W], bf16, name="scrw")
                    nc.vector.scalar_tensor_tensor(
                        scrw[:], eew[:], 1.0, negw_sb[:, mb * W:(mb + 1) * W],
                        OP.mult, OP.mult, accum_out=scol[:, 2:3])
                    mq = wpool.tile([128, W], f32, name="mq")
                    nc.vector.scalar_tensor_tensor(
                        mq[:], qp[:], CP, posw_sb[:, mb * W:(mb + 1) * W],
                        OP.add, OP.mult)
                    nc.vector.reduce_max(mt_all[:, mb:mb + 1], mq[:], axis=AX.X)
                    nmt = vpool.tile([128, 1], f32, name="nmt")
                    nc.vector.tensor_scalar(out=nmt[:], in0=mt_all[:, mb:mb + 1],
                                            scalar1=-1.0, scalar2=None,
                                            op0=OP.mult)
                    epw = wpool.tile([128, W], f32, name="epw")
                    nc.scalar.activation(epw[:], mq[:], AF.Exp, bias=nmt[:],
                                         scale=1.0,
                                         accum_out=sp_all[:, mb:mb + 1])
                    # ---- bulk masked exp: big segments skipping window ----
                    nc.scalar.activation(ee_all[:, we:B], ll_all[:, we:B],
                                         AF.Exp, bias=-XSH, scale=1.0,
                                         accum_out=scol[:, 0:1])
                    if wb > 0:
                        nc.scalar.activation(ee_all[:, 0:wb], ll_all[:, 0:wb],
                                             AF.Exp, bias=-XSH, scale=1.0,
                                             accum_out=scol[:, 1:2])
                        nc.vector.reduce_sum(sn_all[:, mb:mb + 1],
                                             scol[:, 0:3], axis=AX.X)
                    else:
                        # scol[:,1] unwritten: sum cols {0,2}
                        nc.vector.tensor_tensor(out=sn_all[:, mb:mb + 1],
                                                in0=scol[:, 0:1],
                                                in1=scol[:, 2:3], op=OP.add)
                nc.sync.dma_start(OUT[:, 0:NMB], sn_all[:])
                nc.sync.dma_start(OUT[:, NMB:2 * NMB], sp_all[:])
                nc.sync.dma_start(OUT[:, 2 * NMB:3 * NMB], mt_all[:])

    nc.compile()
    return nc


def _host_prep(feat, label):
    feat = np.asarray(feat, dtype=np.float32)
    label = np.asarray(label).astype(np.int64).ravel()
    order = np.argsort(label, kind='stable')
    labs = label[order]
    f = feat[order]
    nrm = np.maximum(np.sqrt((f * f).sum(1, dtype=np.float32)), 1e-12).astype(np.float32)
    g = (f * (4.0 / nrm)[:, None]).astype(np.float32)
    gT = np.ascontiguousarray(g.T)  # [128, B]

    cnt = np.bincount(labs, minlength=512)
    cnt_row = cnt[labs]                       # class size per sorted row
    valid = (cnt_row >= 2) & (B - cnt_row >= 1)
    n_valid = max(int(valid.sum()), 1)

    negeye = (-16.0 * np.eye(128, dtype=np.float32))
    ident = np.eye(128, dtype=np.float32)

    import ml_dtypes
    in_maps, ok = [], True
    for c in range(M):
        shift = ROT - c * RB
        gT_c = np.roll(gT, shift, axis=1)
        labs_c = np.roll(labs, shift)
        posw = np.zeros((RB, W), dtype=np.float32)
        negw = np.zeros((RB, W), dtype=np.float32)
        for mb in range(NMB):
            rl = labs_c[ROT + mb * 128: ROT + (mb + 1) * 128]       # row labels
            wl = labs_c[mb * 128: mb * 128 + W]                     # window labels
            same = (rl[:, None] == wl[None, :])
            # coverage: window must contain every same-class column
            if not (same.sum(1) == cnt[rl]).all():
                ok = False
            pos = same.copy()
            pos[np.arange(128), np.arange(128) + ROT] = False       # drop diagonal
            r0 = mb * 128
            posw[r0:r0 + 128] = pos
            negw[r0:r0 + 128] = ~same
        in_maps.append({
            "gt": gT_c.astype(np.float32),
            "posw": posw.astype(ml_dtypes.bfloat16),
            "negw": negw.astype(ml_dtypes.bfloat16),
            "negeye": negeye,
            "ident": ident,
        })
    return in_maps, n_valid, ok


def _numpy_fallback(feat, label):
    feat = np.asarray(feat, dtype=np.float32)
    label = np.asarray(label).astype(np.int64).ravel()
    nrm = np.maximum(np.linalg.norm(feat, axis=1, keepdims=True), 1e-12)
    f = feat / nrm
    sim = (f @ f.T).astype(np.float32)
    same = label[:, None] == label[None, :]
    eye = np.eye(B, dtype=bool)
    pos_m, neg_m = same & ~eye, ~same
    ap = np.maximum(1.25 - sim, 0)
    an = np.maximum(sim + 0.25, 0)
    lp = np.where(pos_m, -256.0 * ap * (sim - 0.75), -1e9)
    ln_ = np.where(neg_m, 256.0 * an * (sim - 0.25), -1e9)

    def lse(x):
        m = x.max(1, keepdims=True)
        return (m + np.log(np.exp(x - m).sum(1, keepdims=True)))[:, 0]
    val = pos_m.any(1) & neg_m.any(1)
    x = lse(lp) + lse(ln_)
    loss = np.maximum(x, 0) + np.log1p(np.exp(-np.abs(x)))
    return np.float32(np.where(val, loss, 0.0).sum() / max(int(val.sum()), 1))


def _host_finish(res, n_valid):
    total = 0.0
    for c in range(M):
        o = np.asarray(res.results[c]["oloss"], dtype=np.float64)
        sn, sp, mt = o[:, 0:NMB], o[:, NMB:2 * NMB], o[:, 2 * NMB:3 * NMB]
        x = np.log(sp) + np.log(sn) + mt + (LSE_N_SH - CP - 16.0)
        total += np.logaddexp(0.0, x).sum()
    return np.float32(total / n_valid)


def kernel(feat, label):
    from concourse import bass_utils
    in_maps, n_valid, ok = _host_prep(feat, label)
    if not ok:
        return _numpy_fallback(feat, label)
    if 'nc' not in _CACHE:
        _CACHE['nc'] = _build(repeat=1)
    res = bass_utils.run_bass_kernel_spmd(_CACHE['nc'], in_maps,
                                          core_ids=list(range(M)))
    _CACHE['last'] = res
    return _host_finish(res, n_valid)
